# revision 15
# baseline (speedup 1.0000x reference)
"""Trainium2 Bass kernel for causal multi-head attention (B=4, N=2048, DIM=1024, H=16, DH=64).

Sharding: 8 cores = (batch, head-group) pairs. Core c handles batch c//2 and
heads (c%2)*8 .. (c%2)*8+7.  Each core computes QKV projection for its 8 heads,
causal flash-attention, and a partial output projection (its heads' rows of
w_out).  The host sums the two partial outputs per batch and adds b_out.

Device-side layout choices (per core):
  - x is fed pre-transposed as xT [DIM, N] bf16 (host prep), so the QKV
    projection contraction (over DIM) sits on partitions with no on-device
    transpose.
  - Q^T, K^T computed as [head_dim, tok] (weights-stationary matmuls) so that
    scores can be computed directly as S^T = K^T.T @ Q^T with contraction dh=64.
  - Heads are processed in PAIRS: head 2c on SBUF partitions 0-63 of chunk c,
    head 2c+1 on partitions 64-127.  The two scores matmuls of a round are
    issued back-to-back on PE row tiles (0,0) and (64,0) (64x128 mode) writing
    adjacent PSUM banks, so they stream concurrently on the two array halves.
  - Scores tile per round: psAB [128 k-tok, 1024] = head A cols 0:512, head B
    cols 512:1024 (exactly 2 PSUM banks).  One exp per round (3D AP narrows
    the diagonal rounds to the live columns of both halves).
  - Rounds are emitted in PAIRS (4 scores matmuls, then 4 AV matmuls) to halve
    PE tile-mode switches; AVs lag one round-pair so the PE queue never
    head-of-line blocks on the ACT exp latency.
  - Softmax denominator comes free by augmenting V with a ones column:
    O^T_aug = [V | 1].T @ exp(S^T), M=65.
  - Causal masking: multiply exp(S^T) by precomputed 0/1 bf16 tiles on the
    diagonal blocks only (exp of a finite garbage score times 0 is exactly 0).
  - Key-padding mask folds into V_aug: V_aug row k scaled by mask[k] zeroes both
    numerator and denominator contributions of masked keys.
  - Projection work (v_proj / qk_proj / out_proj psum groups) is sprinkled as
    FILLER between attention round-pairs so the PE never idles while the ACT
    exp is the per-round critical path; attention starts ~25us into the run
    instead of ~60us.
  - Engine balance: exp + V-evac on ACT; QK-evac, y-evac, causal masks, recip,
    OT-normalize mults on DVE; rowsum copies + partition broadcasts on gpsimd.
  - Dummy warm-up matmuls at program start overlap the input DMAs and ramp the
    PE clock out of its low p-state before real work arrives.
"""

import numpy as np
import ml_dtypes

B, N, DIM, H, DH = 4, 2048, 1024, 16, 64
HPC = 8            # heads per core
HD = HPC * DH      # 512 head dims per core
NCORES = 8
BF16 = ml_dtypes.bfloat16

TOK_TILE = 128     # k-token tile (partition dim of S^T)
QCHUNK = 512       # q-token chunk (free dim of S^T)
NKT = N // TOK_TILE       # 16 k tiles
NQC = N // QCHUNK         # 4 q chunks
NQT = N // 128            # 16 q tiles (out-projection)
DCH = DIM // 128          # 8 contraction chunks over DIM
VROW = HPC * (DH + 1)     # 520: V_aug row elems per k-tile

_CACHE = {}


def _build_program():
    from contextlib import ExitStack
    import concourse.bass as bass
    import concourse.tile as tile
    from concourse import bacc, mybir

    dt = mybir.dt
    f32 = dt.float32
    bf16 = dt.bfloat16
    Exp = mybir.ActivationFunctionType.Exp
    Copy = mybir.ActivationFunctionType.Copy

    nc = bacc.Bacc("TRN2", target_bir_lowering=False, debug=False,
                   enable_asserts=False, num_devices=NCORES)

    xT = nc.dram_tensor("xT", [DIM, N], bf16, kind="ExternalInput").ap()
    wq = nc.dram_tensor("wq", [DIM, HD], bf16, kind="ExternalInput").ap()
    wk = nc.dram_tensor("wk", [DIM, HD], bf16, kind="ExternalInput").ap()
    wv = nc.dram_tensor("wv", [DIM, HD], bf16, kind="ExternalInput").ap()
    wo = nc.dram_tensor("wo", [HD, DIM], bf16, kind="ExternalInput").ap()
    kpm = nc.dram_tensor("kpm", [N, 1], f32, kind="ExternalInput").ap()
    cmask_d = nc.dram_tensor("cmask", [4 * 128, QCHUNK], bf16,
                             kind="ExternalInput").ap()
    out_d = nc.dram_tensor("out", [N, DIM], f32, kind="ExternalOutput").ap()

    with tile.TileContext(nc) as tc, ExitStack() as ctx:
        const = ctx.enter_context(tc.tile_pool(name="const", bufs=1))
        p_sbp = ctx.enter_context(tc.tile_pool(name="p_sbp", bufs=17))
        miscp = ctx.enter_context(tc.tile_pool(name="miscp", bufs=3))
        outp = ctx.enter_context(tc.tile_pool(name="outp", bufs=3))
        mm_ps = ctx.enter_context(tc.tile_pool(name="mm_ps", bufs=2, space="PSUM"))
        s_ps = ctx.enter_context(tc.tile_pool(name="s_ps", bufs=2, space="PSUM"))
        o_ps = ctx.enter_context(tc.tile_pool(name="o_ps", bufs=2, space="PSUM"))

        # ---- persistent SBUF tensors ----
        xT_sb = [const.tile([128, N], bf16, name=f"xTsb{c}") for c in range(DCH)]
        wq_sb = [const.tile([128, HD], bf16, name=f"wqsb{c}") for c in range(DCH)]
        wk_sb = [const.tile([128, HD], bf16, name=f"wksb{c}") for c in range(DCH)]
        wv_sb = [const.tile([128, HD], bf16, name=f"wvsb{c}") for c in range(DCH)]
        wo_sb = [const.tile([128, DIM], bf16, name=f"wosb{c}") for c in range(4)]
        # Q^T / K^T packed: chunk c holds heads 2c (parts 0-63) and 2c+1 (64-127)
        QT = [const.tile([128, N], bf16, name=f"QTsb{c}") for c in range(4)]
        KT = [const.tile([128, N], bf16, name=f"KTsb{c}") for c in range(4)]
        # V_aug: per k-tile block of 8*(64+1) cols
        V = const.tile([128, NKT * VROW], bf16, name="Vsb")
        # O^T packed like QT/KT
        OT = [const.tile([128, N], bf16, name=f"OTsb{c}") for c in range(4)]
        cmask = const.tile([128, 4 * QCHUNK], bf16, name="cmasksb")
        # key-padding mask: col t = mask[t*128 + p]
        kpm_sb = const.tile([128, NKT], f32, name="kpmsb")
        warm = const.tile([128, 512], bf16, name="warm")

        sync = nc.sync
        sync.dma_start(
            kpm_sb.rearrange("p (t one) -> p t one", one=1),
            kpm.rearrange("(t p) one -> p t one", p=128),
        )

        # ---- PE warm-up: ramp the clock out of the low p-state while the
        # input DMAs run.  Reads a zeroed SBUF tile, output never consumed.
        nc.vector.memset(warm[:], 0)
        wps = s_ps.tile([128, 512], f32, tag="s", name="warmps")
        for i in range(8):
            nc.tensor.matmul(wps[:], warm[:, 0:128], warm[:],
                             start=(i == 0), stop=(i == 7))

        # ---- input DMAs, finely staged so qk(0,0) unblocks at ~4us:
        # xT quarter 0 + wq/wk col-block 0 first, then wv (for v units),
        # then the remaining blocks in need order ----
        for c in range(DCH):
            sync.dma_start(xT_sb[c][:, 0:512], xT[c * 128:(c + 1) * 128, 0:512])
        for c in range(DCH):
            sync.dma_start(wq_sb[c][:, 0:128], wq[c * 128:(c + 1) * 128, 0:128])
            sync.dma_start(wk_sb[c][:, 0:128], wk[c * 128:(c + 1) * 128, 0:128])
        for c in range(DCH):
            sync.dma_start(wv_sb[c][:], wv[c * 128:(c + 1) * 128, :])
        # cmask DRAM row r*128+k, col q  ->  SBUF part k, col r*512+q
        # (needed by the first diagonal mask-mults at ~8us)
        sync.dma_start(
            cmask.rearrange("p (r q) -> p r q", r=4),
            cmask_d.rearrange("(r p) q -> p r q", p=128),
        )
        for q in range(1, 4):
            for c in range(DCH):
                sync.dma_start(xT_sb[c][:, q * 512:(q + 1) * 512],
                               xT[c * 128:(c + 1) * 128, q * 512:(q + 1) * 512])
        for c in range(DCH):
            sync.dma_start(wq_sb[c][:, 128:512], wq[c * 128:(c + 1) * 128, 128:512])
            sync.dma_start(wk_sb[c][:, 128:512], wk[c * 128:(c + 1) * 128, 128:512])
        for c in range(4):
            sync.dma_start(wo_sb[c][:], wo[c * 128:(c + 1) * 128, :])

        # ---- filler units: one PSUM group each ----
        def v_unit(kt):
            kpm_t = kpm_sb[:, kt:kt + 1]
            ps = mm_ps.tile([128, 512], f32, tag="mm", name="ps")
            for c in range(DCH):
                nc.tensor.matmul(
                    ps[:], xT_sb[c][:, kt * 128:(kt + 1) * 128], wv_sb[c][:],
                    start=(c == 0), stop=(c == DCH - 1))
            vblk = V[:, kt * VROW:(kt + 1) * VROW].rearrange(
                "p (h c) -> p h c", c=DH + 1)
            # data cols, scaled by key-padding mask (ACT: fast PSUM reads)
            nc.scalar.activation(
                vblk[:, :, 0:DH],
                ps.rearrange("p (h c) -> p h c", c=DH),
                Copy, scale=kpm_t[:, 0:1])
            # ones column = mask value (free-dim stride-0 broadcast read)
            nc.vector.tensor_copy(vblk[:, :, DH:DH + 1].squeeze(),
                                  kpm_t[:, 0:1].broadcast_to([128, HPC]))

        def qk_unit(c, tcx, which):
            tsl = slice(tcx * QCHUNK, (tcx + 1) * QCHUNK)
            w_sb, dst = (wq_sb, QT) if which == "q" else (wk_sb, KT)
            ps = mm_ps.tile([128, 512], f32, tag="mm", name="psqk")
            for d in range(DCH):
                nc.tensor.matmul(
                    ps[:], w_sb[d][:, c * 128:(c + 1) * 128],
                    xT_sb[d][:, tsl],
                    start=(d == 0), stop=(d == DCH - 1))
            nc.vector.tensor_copy(dst[c][:, tsl], ps[:])

        def out_unit(qt):
            y_sb = outp.tile([128, DIM], f32, tag="y", name="y_sb")
            for oc in range(2):
                psy = mm_ps.tile([128, 512], f32, tag="mm", name="psy")
                for cc in range(4):
                    nc.tensor.matmul(
                        psy[:], OT[cc][:, qt * 128:(qt + 1) * 128],
                        wo_sb[cc][:, oc * 512:(oc + 1) * 512],
                        start=(cc == 0), stop=(cc == 3))
                nc.vector.tensor_copy(y_sb[:, oc * 512:(oc + 1) * 512], psy[:])
            sync.dma_start(out_d[qt * 128:(qt + 1) * 128, :], y_sb[:])

        def attend_pair(c, qc, fillers=()):
            """Both heads of chunk c over q-chunk qc.  Rounds emitted in pairs
            (4 scores matmuls in 64-row mode, then 4 AV matmuls in 128-row
            mode, AVs lagged one round-pair); proj filler units interleaved."""
            hA, hB = 2 * c, 2 * c + 1
            ktA, ktB = KT[c][0:64, :], KT[c][64:128, :]
            qtA, qtB = QT[c][0:64, :], QT[c][64:128, :]
            qsl = slice(qc * QCHUNK, (qc + 1) * QCHUNK)
            psoA = o_ps.tile([DH + 1, 512], f32, tag="o", name="psoA")
            psoB = o_ps.tile([DH + 1, 512], f32, tag="o", name="psoB")
            nkt = 4 * qc + 4
            fillers = list(fillers)
            n_rp = nkt // 2

            def emit_scores(kt):
                off = max(0, (kt - 4 * qc) * 128)
                ps = s_ps.tile([128, 1024], f32, tag="s", name="ps")
                nc.tensor.matmul(
                    ps[:, off:512],
                    ktA[:, kt * 128:(kt + 1) * 128],
                    qtA[:, qc * QCHUNK + off:(qc + 1) * QCHUNK],
                    start=True, stop=True)
                nc.tensor.matmul(
                    ps[:, 512 + off:1024],
                    ktB[:, kt * 128:(kt + 1) * 128],
                    qtB[:, qc * QCHUNK + off:(qc + 1) * QCHUNK],
                    start=True, stop=True)
                p2 = p_sbp.tile([128, 1024], bf16, tag="p", name="p2")
                if off > 0:
                    # diagonal round: exp only the live columns of both halves
                    ps3 = ps.rearrange("p (h q) -> p h q", h=2)[:, :, off:512]
                    p23 = p2.rearrange("p (h q) -> p h q", h=2)[:, :, off:512]
                    nc.scalar.activation(p23, ps3, Exp)
                else:
                    nc.scalar.activation(p2[:], ps[:], Exp)
                if kt >= 4 * qc:
                    r = kt - 4 * qc
                    cm = cmask[:, r * QCHUNK + off:(r + 1) * QCHUNK]
                    nc.vector.tensor_mul(p2[:, off:512], p2[:, off:512], cm)
                    nc.vector.tensor_mul(p2[:, 512 + off:1024],
                                         p2[:, 512 + off:1024], cm)
                return p2

            def emit_av(kt, p2, pso, h, half):
                off = max(0, (kt - 4 * qc) * 128)
                st, sp = (kt == 0), (kt == nkt - 1)
                nc.tensor.matmul(
                    pso[:, off:512],
                    V[:, kt * VROW + h * (DH + 1):
                       kt * VROW + (h + 1) * (DH + 1)],
                    p2[:, half * 512 + off:half * 512 + 512],
                    start=st, stop=sp, skip_group_check=True)

            def normalize(pso, po):
                # O^T[0:64] * (1 / rowsum row 64)
                rs = miscp.tile([1, 512], f32, tag="rs", name="rs")
                nc.vector.tensor_copy(rs[:], pso[DH:DH + 1, :])
                rc = miscp.tile([1, 512], f32, tag="rc", name="rc")
                nc.vector.reciprocal_approx_fast(rc[:], rs[:])
                bc = miscp.tile([64, 512], f32, tag="bc", name="bc")
                nc.gpsimd.partition_broadcast(bc[:], rc[:])
                if po == 0:
                    nc.vector.tensor_mul(OT[c][0:64, qsl],
                                         pso[0:DH, :], bc[:])
                else:
                    otmp = miscp.tile([64, 512], bf16, tag="otmp", bufs=3,
                                      name="otmp")
                    nc.vector.tensor_mul(otmp[:], pso[0:DH, :], bc[:])
                    # partition shift 0->64 needs a DMA, engines can't shift
                    sync.dma_start(OT[c][64:128, qsl], otmp[:])

            # qc-phase batching: ALL scores rounds (one 64-row-mode phase,
            # paced by ACT exp, fillers absorb the PE slack), then ALL AV
            # matmuls (one 128-row-mode phase) -> few mode switches per qc.
            # Head A's AVs complete first so its normalize chain (DVE/gpsimd)
            # hides under head B's AV stream.
            pending = []
            done_f = 0
            for kt in range(nkt):
                pending.append((kt, emit_scores(kt)))
                want = (kt + 1) * len(fillers) // nkt
                fired = done_f < want
                while done_f < want:
                    fillers[done_f]()
                    done_f += 1
                if not fired and kt >= 3:
                    # keep-warm: the scores phase is exp-paced; without PE
                    # work the DVFS drops the array clock (matmuls at ~1.6GHz
                    # instead of 2.4).  Two dummy 64-row-mode matmuls into
                    # psoA (erased by the AV chain's start=True reset) hold
                    # the clock with no mode switch and no critical-path cost.
                    for _ in range(2):
                        nc.tensor.matmul(psoA[:, 0:512], warm[0:64, 0:65],
                                         warm[0:64, :], start=True, stop=True,
                                         skip_group_check=True)
            # B first: its longer normalize chain (extra DMA partition-shift)
            # hides under A's AV stream; A's shorter chain is tail-exposed.
            for kt, p2 in pending:
                emit_av(kt, p2, psoB, hB, 1)
            normalize(psoB, 64)
            for kt, p2 in pending:
                emit_av(kt, p2, psoA, hA, 0)
            normalize(psoA, 0)

        # ---- global schedule: attention starts after qk(0,0)+v(0..3);
        # all remaining proj groups ride as filler inside attention ----
        F = {}
        for cc in range(4):
            for t in range(NQC):
                F[f"q{cc}{t}"] = (lambda cc=cc, t=t: qk_unit(cc, t, "q"))
                F[f"k{cc}{t}"] = (lambda cc=cc, t=t: qk_unit(cc, t, "k"))
        for kt in range(NKT):
            F[f"v{kt}"] = (lambda kt=kt: v_unit(kt))
        for qt in range(NQT):
            F[f"o{qt}"] = (lambda qt=qt: out_unit(qt))

        qk_unit(0, 0, "q"); qk_unit(0, 0, "k")
        for kt in range(4):
            v_unit(kt)

        plan = {
            (0, 0): ["v4", "v5", "v6", "v7", "q01", "k01"],
            (0, 1): ["v8", "v9", "v10", "v11", "q02", "k02"],
            (0, 2): ["v12", "v13", "v14", "v15", "q03", "k03"],
            (0, 3): ["q10", "k10", "q11", "k11"],
            (1, 0): ["q12", "k12"],
            (1, 1): ["q13", "k13"],
            (1, 2): ["q20", "k20", "q21", "k21"],
            (1, 3): ["q22", "k22", "q23", "k23"],
            (2, 0): [],
            (2, 1): ["q30", "k30"],
            (2, 2): ["q31", "k31", "q32", "k32"],
            (2, 3): ["q33", "k33"],
            (3, 0): [],
            (3, 1): ["o0", "o1", "o2", "o3"],
            (3, 2): ["o4", "o5", "o6", "o7"],
            (3, 3): ["o8", "o9", "o10", "o11"],
        }
        for c in range(4):
            for qc in range(NQC):
                attend_pair(c, qc, [F[n] for n in plan[(c, qc)]])
        for qt in range(12, 16):
            out_unit(qt)

    nc.compile()
    return nc


def _get_program():
    if "nc" not in _CACHE:
        _CACHE["nc"] = _build_program()
    return _CACHE["nc"]


def _prep_inputs(x, mask, w_qkv, w_out):
    """Build the 8 per-core input maps (host-side sharding)."""
    scale = DH ** -0.5
    # causal keep-mask patterns for the 4 diagonal k-tiles of a 512 q-chunk
    k_idx = np.arange(128)[:, None]
    q_idx = np.arange(QCHUNK)[None, :]
    cm = np.concatenate(
        [(q_idx >= r * 128 + k_idx) for r in range(4)], axis=0
    ).astype(BF16)  # [512, 512]

    xT = [np.ascontiguousarray(x[b].T).astype(BF16) for b in range(B)]
    in_maps = []
    for core in range(NCORES):
        b, hg = core // 2, core % 2
        cs = slice(hg * HD, (hg + 1) * HD)
        wq_s = (w_qkv[:, 0 * DIM:1 * DIM][:, cs] * scale).astype(BF16)
        wk_s = w_qkv[:, 1 * DIM:2 * DIM][:, cs].astype(BF16)
        wv_s = w_qkv[:, 2 * DIM:3 * DIM][:, cs].astype(BF16)
        wo_s = np.ascontiguousarray(w_out[cs, :]).astype(BF16)
        kpm = mask[b].astype(np.float32).reshape(N, 1)
        in_maps.append({
            "xT": xT[b], "wq": wq_s, "wk": wk_s, "wv": wv_s, "wo": wo_s,
            "kpm": np.ascontiguousarray(kpm), "cmask": cm,
        })
    return in_maps


def kernel(x, mask, w_qkv, w_out, b_out, _trace=False):
    from concourse import bass_utils

    x = np.asarray(x, dtype=np.float32)
    mask = np.asarray(mask)
    w_qkv = np.asarray(w_qkv, dtype=np.float32)
    w_out = np.asarray(w_out, dtype=np.float32)
    b_out = np.asarray(b_out, dtype=np.float32)

    nc = _get_program()
    in_maps = _prep_inputs(x, mask, w_qkv, w_out)
    res = bass_utils.run_bass_kernel_spmd(
        nc, in_maps, core_ids=list(range(NCORES)), trace=_trace)

    out = np.empty((B, N, DIM), dtype=np.float32)
    for b in range(B):
        out[b] = res.results[2 * b]["out"] + res.results[2 * b + 1]["out"] + b_out
    if _trace:
        return out, res
    return out


# revision 18
# speedup vs baseline: 1.1946x; 1.1946x over previous
"""Trainium2 Bass kernel for causal multi-head attention (B=4, N=2048, DIM=1024, H=16, DH=64).

Sharding: 8 cores = (batch, head-group) pairs. Core c handles batch c//2 and
heads (c%2)*8 .. (c%2)*8+7.  Each core computes QKV projection for its 8 heads,
causal flash-attention, and a partial output projection (its heads' rows of
w_out).  The host sums the two partial outputs per batch and adds b_out.

Device-side layout choices (per core):
  - x is fed pre-transposed as xT [DIM, N] bf16 (host prep), so the QKV
    projection contraction (over DIM) sits on partitions with no on-device
    transpose.
  - Q^T, K^T computed as [head_dim, tok] (weights-stationary matmuls) so that
    scores can be computed directly as S^T = K^T.T @ Q^T with contraction dh=64.
  - Heads are processed in PAIRS: head 2c on SBUF partitions 0-63 of chunk c,
    head 2c+1 on partitions 64-127.  The two scores matmuls of a round are
    issued back-to-back on PE row tiles (0,0) and (64,0) (64x128 mode) writing
    adjacent PSUM banks, so they stream concurrently on the two array halves.
  - Scores tile per round: psAB [128 k-tok, 1024] = head A cols 0:512, head B
    cols 512:1024 (exactly 2 PSUM banks).  One exp per round (3D AP narrows
    the diagonal rounds to the live columns of both halves).
  - Rounds are emitted in PAIRS (4 scores matmuls, then 4 AV matmuls) to halve
    PE tile-mode switches; AVs lag one round-pair so the PE queue never
    head-of-line blocks on the ACT exp latency.
  - Softmax denominator comes free by augmenting V with a ones column:
    O^T_aug = [V | 1].T @ exp(S^T), M=65.
  - Causal masking: multiply exp(S^T) by precomputed 0/1 bf16 tiles on the
    diagonal blocks only (exp of a finite garbage score times 0 is exactly 0).
  - Key-padding mask folds into V_aug: V_aug row k scaled by mask[k] zeroes both
    numerator and denominator contributions of masked keys.
  - Projection work (v_proj / qk_proj / out_proj psum groups) is sprinkled as
    FILLER between attention round-pairs so the PE never idles while the ACT
    exp is the per-round critical path; attention starts ~25us into the run
    instead of ~60us.
  - Engine balance: exp + V-evac on ACT; QK-evac, y-evac, causal masks, recip,
    OT-normalize mults on DVE; rowsum copies + partition broadcasts on gpsimd.
  - Dummy warm-up matmuls at program start overlap the input DMAs and ramp the
    PE clock out of its low p-state before real work arrives.
"""

import numpy as np
import ml_dtypes

B, N, DIM, H, DH = 4, 2048, 1024, 16, 64
HPC = 8            # heads per core
HD = HPC * DH      # 512 head dims per core
NCORES = 8
BF16 = ml_dtypes.bfloat16

TOK_TILE = 128     # k-token tile (partition dim of S^T)
QCHUNK = 512       # q-token chunk (free dim of S^T)
NKT = N // TOK_TILE       # 16 k tiles
NQC = N // QCHUNK         # 4 q chunks
NQT = N // 128            # 16 q tiles (out-projection)
DCH = DIM // 128          # 8 contraction chunks over DIM
VROW = HPC * (DH + 1)     # 520: V_aug row elems per k-tile

_CACHE = {}


def _build_program():
    from contextlib import ExitStack
    import concourse.bass as bass
    import concourse.tile as tile
    from concourse import bacc, mybir

    dt = mybir.dt
    f32 = dt.float32
    bf16 = dt.bfloat16
    Exp = mybir.ActivationFunctionType.Exp
    Copy = mybir.ActivationFunctionType.Copy

    nc = bacc.Bacc("TRN2", target_bir_lowering=False, debug=False,
                   enable_asserts=False, num_devices=NCORES)

    xT = nc.dram_tensor("xT", [DIM, N], bf16, kind="ExternalInput").ap()
    wq = nc.dram_tensor("wq", [DIM, HD], bf16, kind="ExternalInput").ap()
    wk = nc.dram_tensor("wk", [DIM, HD], bf16, kind="ExternalInput").ap()
    wv = nc.dram_tensor("wv", [DIM, HD], bf16, kind="ExternalInput").ap()
    wo = nc.dram_tensor("wo", [HD, DIM], bf16, kind="ExternalInput").ap()
    kpm = nc.dram_tensor("kpm", [N, 1], f32, kind="ExternalInput").ap()
    cmask_d = nc.dram_tensor("cmask", [4 * 128, QCHUNK], bf16,
                             kind="ExternalInput").ap()
    out_d = nc.dram_tensor("out", [N, DIM], f32, kind="ExternalOutput").ap()

    with tile.TileContext(nc) as tc, ExitStack() as ctx:
        const = ctx.enter_context(tc.tile_pool(name="const", bufs=1))
        p_sbp = ctx.enter_context(tc.tile_pool(name="p_sbp", bufs=17))
        miscp = ctx.enter_context(tc.tile_pool(name="miscp", bufs=3))
        outp = ctx.enter_context(tc.tile_pool(name="outp", bufs=3))
        mm_ps = ctx.enter_context(tc.tile_pool(name="mm_ps", bufs=2, space="PSUM"))
        s_ps = ctx.enter_context(tc.tile_pool(name="s_ps", bufs=2, space="PSUM"))
        o_ps = ctx.enter_context(tc.tile_pool(name="o_ps", bufs=2, space="PSUM"))

        # ---- persistent SBUF tensors ----
        xT_sb = [const.tile([128, N], bf16, name=f"xTsb{c}") for c in range(DCH)]
        wq_sb = [const.tile([128, HD], bf16, name=f"wqsb{c}") for c in range(DCH)]
        wk_sb = [const.tile([128, HD], bf16, name=f"wksb{c}") for c in range(DCH)]
        wv_sb = [const.tile([128, HD], bf16, name=f"wvsb{c}") for c in range(DCH)]
        wo_sb = [const.tile([128, DIM], bf16, name=f"wosb{c}") for c in range(4)]
        # Q^T / K^T packed: chunk c holds heads 2c (parts 0-63) and 2c+1 (64-127)
        QT = [const.tile([128, N], bf16, name=f"QTsb{c}") for c in range(4)]
        KT = [const.tile([128, N], bf16, name=f"KTsb{c}") for c in range(4)]
        # V_aug: per k-tile block of 8*(64+1) cols
        V = const.tile([128, NKT * VROW], bf16, name="Vsb")
        # O^T packed like QT/KT
        OT = [const.tile([128, N], bf16, name=f"OTsb{c}") for c in range(4)]
        cmask = const.tile([128, 4 * QCHUNK], bf16, name="cmasksb")
        # key-padding mask: col t = mask[t*128 + p]
        kpm_sb = const.tile([128, NKT], f32, name="kpmsb")
        warm = const.tile([128, 512], bf16, name="warm")

        sync = nc.sync
        sync.dma_start(
            kpm_sb.rearrange("p (t one) -> p t one", one=1),
            kpm.rearrange("(t p) one -> p t one", p=128),
        )

        # ---- PE warm-up: ramp the clock out of the low p-state while the
        # input DMAs run.  Reads a zeroed SBUF tile, output never consumed.
        nc.vector.memset(warm[:], 0)
        wps = s_ps.tile([128, 512], f32, tag="s", name="warmps")
        for i in range(8):
            nc.tensor.matmul(wps[:], warm[:, 0:128], warm[:],
                             start=(i == 0), stop=(i == 7))

        # ---- input DMAs, finely staged so qk(0,0) unblocks at ~4us:
        # xT quarter 0 + wq/wk col-block 0 first, then wv (for v units),
        # then the remaining blocks in need order ----
        for c in range(DCH):
            sync.dma_start(xT_sb[c][:, 0:512], xT[c * 128:(c + 1) * 128, 0:512])
        for c in range(DCH):
            sync.dma_start(wq_sb[c][:, 0:128], wq[c * 128:(c + 1) * 128, 0:128])
            sync.dma_start(wk_sb[c][:, 0:128], wk[c * 128:(c + 1) * 128, 0:128])
        for c in range(DCH):
            sync.dma_start(wv_sb[c][:], wv[c * 128:(c + 1) * 128, :])
        # cmask DRAM row r*128+k, col q  ->  SBUF part k, col r*512+q
        # (needed by the first diagonal mask-mults at ~8us)
        sync.dma_start(
            cmask.rearrange("p (r q) -> p r q", r=4),
            cmask_d.rearrange("(r p) q -> p r q", p=128),
        )
        for q in range(1, 4):
            for c in range(DCH):
                sync.dma_start(xT_sb[c][:, q * 512:(q + 1) * 512],
                               xT[c * 128:(c + 1) * 128, q * 512:(q + 1) * 512])
        for c in range(DCH):
            sync.dma_start(wq_sb[c][:, 128:512], wq[c * 128:(c + 1) * 128, 128:512])
            sync.dma_start(wk_sb[c][:, 128:512], wk[c * 128:(c + 1) * 128, 128:512])
        for c in range(4):
            sync.dma_start(wo_sb[c][:], wo[c * 128:(c + 1) * 128, :])

        # ---- filler units: one PSUM group each ----
        def v_unit(kt):
            kpm_t = kpm_sb[:, kt:kt + 1]
            ps = mm_ps.tile([128, 512], f32, tag="mm", name="ps")
            for c in range(DCH):
                nc.tensor.matmul(
                    ps[:], xT_sb[c][:, kt * 128:(kt + 1) * 128], wv_sb[c][:],
                    start=(c == 0), stop=(c == DCH - 1))
            vblk = V[:, kt * VROW:(kt + 1) * VROW].rearrange(
                "p (h c) -> p h c", c=DH + 1)
            # data cols, scaled by key-padding mask (ACT: fast PSUM reads)
            nc.scalar.activation(
                vblk[:, :, 0:DH],
                ps.rearrange("p (h c) -> p h c", c=DH),
                Copy, scale=kpm_t[:, 0:1])
            # ones column = mask value (free-dim stride-0 broadcast read)
            nc.vector.tensor_copy(vblk[:, :, DH:DH + 1].squeeze(),
                                  kpm_t[:, 0:1].broadcast_to([128, HPC]))

        def qk_unit(c, tcx, which):
            tsl = slice(tcx * QCHUNK, (tcx + 1) * QCHUNK)
            w_sb, dst = (wq_sb, QT) if which == "q" else (wk_sb, KT)
            ps = mm_ps.tile([128, 512], f32, tag="mm", name="psqk")
            for d in range(DCH):
                nc.tensor.matmul(
                    ps[:], w_sb[d][:, c * 128:(c + 1) * 128],
                    xT_sb[d][:, tsl],
                    start=(d == 0), stop=(d == DCH - 1))
            nc.vector.tensor_copy(dst[c][:, tsl], ps[:])

        def out_unit(qt):
            y_sb = outp.tile([128, DIM], f32, tag="y", name="y_sb")
            for oc in range(2):
                psy = mm_ps.tile([128, 512], f32, tag="mm", name="psy")
                for cc in range(4):
                    nc.tensor.matmul(
                        psy[:], OT[cc][:, qt * 128:(qt + 1) * 128],
                        wo_sb[cc][:, oc * 512:(oc + 1) * 512],
                        start=(cc == 0), stop=(cc == 3))
                nc.vector.tensor_copy(y_sb[:, oc * 512:(oc + 1) * 512], psy[:])
            sync.dma_start(out_d[qt * 128:(qt + 1) * 128, :], y_sb[:])

        def attend_pair(c, qc, fillers=()):
            """Both heads of chunk c over q-chunk qc.  Rounds emitted in pairs
            (4 scores matmuls in 64-row mode, then 4 AV matmuls in 128-row
            mode, AVs lagged one round-pair); proj filler units interleaved."""
            hA, hB = 2 * c, 2 * c + 1
            ktA, ktB = KT[c][0:64, :], KT[c][64:128, :]
            qtA, qtB = QT[c][0:64, :], QT[c][64:128, :]
            qsl = slice(qc * QCHUNK, (qc + 1) * QCHUNK)
            psoA = o_ps.tile([DH + 1, 512], f32, tag="o", name="psoA")
            psoB = o_ps.tile([DH + 1, 512], f32, tag="o", name="psoB")
            nkt = 4 * qc + 4
            fillers = list(fillers)
            n_rp = nkt // 2

            def emit_scores(kt, keepwarm=False):
                off = max(0, (kt - 4 * qc) * 128)
                ps = s_ps.tile([128, 1024], f32, tag="s", name="ps")
                if keepwarm:
                    # exp-paced phase with no filler: the PE would idle and
                    # DVFS drops the array clock.  Two dummy 64-row-mode
                    # matmuls into this round's own scores tile (same pool
                    # wait as the real matmuls, so no extra stall) hold the
                    # clock; the real scores overwrite via start=True.  Each
                    # dummy sits on the SAME row tile (T0/T8) as the real
                    # matmul that follows it in its bank — concurrent tiles
                    # must never touch the same PSUM bank.
                    for half in (0, 1):
                        po = half * 64
                        nc.tensor.matmul(
                            ps[:, half * 512:(half + 1) * 512],
                            warm[po:po + 64, 0:128], warm[po:po + 64, :],
                            start=True, stop=True, skip_group_check=True)
                nc.tensor.matmul(
                    ps[:, off:512],
                    ktA[:, kt * 128:(kt + 1) * 128],
                    qtA[:, qc * QCHUNK + off:(qc + 1) * QCHUNK],
                    start=True, stop=True)
                nc.tensor.matmul(
                    ps[:, 512 + off:1024],
                    ktB[:, kt * 128:(kt + 1) * 128],
                    qtB[:, qc * QCHUNK + off:(qc + 1) * QCHUNK],
                    start=True, stop=True)
                p2 = p_sbp.tile([128, 1024], bf16, tag="p", name="p2")
                if off > 0:
                    # diagonal round: exp only the live columns of both halves
                    ps3 = ps.rearrange("p (h q) -> p h q", h=2)[:, :, off:512]
                    p23 = p2.rearrange("p (h q) -> p h q", h=2)[:, :, off:512]
                    nc.scalar.activation(p23, ps3, Exp)
                else:
                    nc.scalar.activation(p2[:], ps[:], Exp)
                if kt >= 4 * qc:
                    r = kt - 4 * qc
                    cm = cmask[:, r * QCHUNK + off:(r + 1) * QCHUNK]
                    nc.vector.tensor_mul(p2[:, off:512], p2[:, off:512], cm)
                    nc.vector.tensor_mul(p2[:, 512 + off:1024],
                                         p2[:, 512 + off:1024], cm)
                return p2

            def emit_av(kt, p2, pso, h, half):
                off = max(0, (kt - 4 * qc) * 128)
                st, sp = (kt == 0), (kt == nkt - 1)
                nc.tensor.matmul(
                    pso[:, off:512],
                    V[:, kt * VROW + h * (DH + 1):
                       kt * VROW + (h + 1) * (DH + 1)],
                    p2[:, half * 512 + off:half * 512 + 512],
                    start=st, stop=sp, skip_group_check=True)

            def normalize(pso, po):
                # O^T[0:64] * (1 / rowsum row 64)
                rs = miscp.tile([1, 512], f32, tag="rs", name="rs")
                nc.vector.tensor_copy(rs[:], pso[DH:DH + 1, :])
                rc = miscp.tile([1, 512], f32, tag="rc", name="rc")
                nc.vector.reciprocal_approx_fast(rc[:], rs[:])
                bc = miscp.tile([64, 512], f32, tag="bc", name="bc")
                nc.gpsimd.partition_broadcast(bc[:], rc[:])
                if po == 0:
                    nc.vector.tensor_mul(OT[c][0:64, qsl],
                                         pso[0:DH, :], bc[:])
                else:
                    otmp = miscp.tile([64, 512], bf16, tag="otmp", bufs=3,
                                      name="otmp")
                    nc.vector.tensor_mul(otmp[:], pso[0:DH, :], bc[:])
                    # partition shift 0->64 needs a DMA, engines can't shift
                    sync.dma_start(OT[c][64:128, qsl], otmp[:])

            # qc-phase batching: ALL scores rounds (one 64-row-mode phase,
            # paced by ACT exp, fillers absorb the PE slack), then ALL AV
            # matmuls (one 128-row-mode phase) -> few mode switches per qc.
            # Head A's AVs complete first so its normalize chain (DVE/gpsimd)
            # hides under head B's AV stream.
            pending = []
            done_f = 0
            for kt in range(nkt):
                want = (kt + 1) * len(fillers) // nkt
                starved = (done_f >= want) and kt >= 1
                pending.append((kt, emit_scores(kt, keepwarm=starved)))
                while done_f < want:
                    fillers[done_f]()
                    done_f += 1
            # B first: its longer normalize chain (extra DMA partition-shift)
            # hides under A's AV stream; A's shorter chain is tail-exposed.
            for kt, p2 in pending:
                emit_av(kt, p2, psoB, hB, 1)
            normalize(psoB, 64)
            for kt, p2 in pending:
                emit_av(kt, p2, psoA, hA, 0)
            normalize(psoA, 0)

        # ---- global schedule: attention starts after qk(0,0)+v(0..3);
        # all remaining proj groups ride as filler inside attention ----
        F = {}
        for cc in range(4):
            for t in range(NQC):
                F[f"q{cc}{t}"] = (lambda cc=cc, t=t: qk_unit(cc, t, "q"))
                F[f"k{cc}{t}"] = (lambda cc=cc, t=t: qk_unit(cc, t, "k"))
        for kt in range(NKT):
            F[f"v{kt}"] = (lambda kt=kt: v_unit(kt))
        for qt in range(NQT):
            F[f"o{qt}"] = (lambda qt=qt: out_unit(qt))

        qk_unit(0, 0, "q"); qk_unit(0, 0, "k")
        for kt in range(4):
            v_unit(kt)

        plan = {
            (0, 0): ["v4", "v5", "v6", "v7", "q01", "k01"],
            (0, 1): ["v8", "v9", "v10", "v11", "q02", "k02"],
            (0, 2): ["v12", "v13", "v14", "v15", "q03", "k03"],
            (0, 3): ["q10", "k10", "q11", "k11"],
            (1, 0): ["q12", "k12"],
            (1, 1): ["q13", "k13"],
            (1, 2): ["q20", "k20", "q21", "k21"],
            (1, 3): ["q22", "k22", "q23", "k23"],
            (2, 0): [],
            (2, 1): ["q30", "k30"],
            (2, 2): ["q31", "k31", "q32", "k32"],
            (2, 3): ["q33", "k33"],
            (3, 0): [],
            (3, 1): ["o0", "o1", "o2", "o3"],
            (3, 2): ["o4", "o5", "o6", "o7"],
            (3, 3): ["o8", "o9", "o10", "o11"],
        }
        for c in range(4):
            for qc in range(NQC):
                attend_pair(c, qc, [F[n] for n in plan[(c, qc)]])
        for qt in range(12, 16):
            out_unit(qt)

    nc.compile()
    return nc


def _get_program():
    if "nc" not in _CACHE:
        _CACHE["nc"] = _build_program()
    return _CACHE["nc"]


def _prep_inputs(x, mask, w_qkv, w_out):
    """Build the 8 per-core input maps (host-side sharding)."""
    scale = DH ** -0.5
    # causal keep-mask patterns for the 4 diagonal k-tiles of a 512 q-chunk
    k_idx = np.arange(128)[:, None]
    q_idx = np.arange(QCHUNK)[None, :]
    cm = np.concatenate(
        [(q_idx >= r * 128 + k_idx) for r in range(4)], axis=0
    ).astype(BF16)  # [512, 512]

    xT = [np.ascontiguousarray(x[b].T).astype(BF16) for b in range(B)]
    in_maps = []
    for core in range(NCORES):
        b, hg = core // 2, core % 2
        cs = slice(hg * HD, (hg + 1) * HD)
        wq_s = (w_qkv[:, 0 * DIM:1 * DIM][:, cs] * scale).astype(BF16)
        wk_s = w_qkv[:, 1 * DIM:2 * DIM][:, cs].astype(BF16)
        wv_s = w_qkv[:, 2 * DIM:3 * DIM][:, cs].astype(BF16)
        wo_s = np.ascontiguousarray(w_out[cs, :]).astype(BF16)
        kpm = mask[b].astype(np.float32).reshape(N, 1)
        in_maps.append({
            "xT": xT[b], "wq": wq_s, "wk": wk_s, "wv": wv_s, "wo": wo_s,
            "kpm": np.ascontiguousarray(kpm), "cmask": cm,
        })
    return in_maps


def kernel(x, mask, w_qkv, w_out, b_out, _trace=False):
    from concourse import bass_utils

    x = np.asarray(x, dtype=np.float32)
    mask = np.asarray(mask)
    w_qkv = np.asarray(w_qkv, dtype=np.float32)
    w_out = np.asarray(w_out, dtype=np.float32)
    b_out = np.asarray(b_out, dtype=np.float32)

    nc = _get_program()
    in_maps = _prep_inputs(x, mask, w_qkv, w_out)
    res = bass_utils.run_bass_kernel_spmd(
        nc, in_maps, core_ids=list(range(NCORES)), trace=_trace)

    out = np.empty((B, N, DIM), dtype=np.float32)
    for b in range(B):
        out[b] = res.results[2 * b]["out"] + res.results[2 * b + 1]["out"] + b_out
    if _trace:
        return out, res
    return out


# revision 23
# speedup vs baseline: 1.2819x; 1.0731x over previous
"""Trainium2 Bass kernel for causal multi-head attention (B=4, N=2048, DIM=1024, H=16, DH=64).

Sharding: 8 cores = (batch, head-group) pairs. Core c handles batch c//2 and
heads (c%2)*8 .. (c%2)*8+7.  Each core computes QKV projection for its 8 heads,
causal flash-attention, and a partial output projection (its heads' rows of
w_out).  The host sums the two partial outputs per batch and adds b_out.

Device-side layout choices (per core):
  - x is fed pre-transposed as xT [DIM, N] bf16 (host prep), so the QKV
    projection contraction (over DIM) sits on partitions with no on-device
    transpose.
  - Q^T, K^T computed as [head_dim, tok] (weights-stationary matmuls) so that
    scores can be computed directly as S^T = K^T.T @ Q^T with contraction dh=64.
  - Heads are processed in PAIRS: head 2c on SBUF partitions 0-63 of chunk c,
    head 2c+1 on partitions 64-127.  The two scores matmuls of a round are
    issued back-to-back on PE row tiles (0,0) and (64,0) (64x128 mode) writing
    adjacent PSUM banks, so they stream concurrently on the two array halves.
  - Scores tile per round: psAB [128 k-tok, 1024] = head A cols 0:512, head B
    cols 512:1024 (exactly 2 PSUM banks).  One exp per round (3D AP narrows
    the diagonal rounds to the live columns of both halves).
  - Rounds are emitted in PAIRS (4 scores matmuls, then 4 AV matmuls) to halve
    PE tile-mode switches; AVs lag one round-pair so the PE queue never
    head-of-line blocks on the ACT exp latency.
  - Softmax denominator comes free by augmenting V with a ones column:
    O^T_aug = [V | 1].T @ exp(S^T), M=65.
  - Causal masking: multiply exp(S^T) by precomputed 0/1 bf16 tiles on the
    diagonal blocks only (exp of a finite garbage score times 0 is exactly 0).
  - Key-padding mask folds into V_aug: V_aug row k scaled by mask[k] zeroes both
    numerator and denominator contributions of masked keys.
  - Projection work (v_proj / qk_proj / out_proj psum groups) is sprinkled as
    FILLER between attention round-pairs so the PE never idles while the ACT
    exp is the per-round critical path; attention starts ~25us into the run
    instead of ~60us.
  - Engine balance: exp + V-evac on ACT; QK-evac, y-evac, causal masks, recip,
    OT-normalize mults on DVE; rowsum copies + partition broadcasts on gpsimd.
  - Dummy warm-up matmuls at program start overlap the input DMAs and ramp the
    PE clock out of its low p-state before real work arrives.
"""

import numpy as np
import ml_dtypes

B, N, DIM, H, DH = 4, 2048, 1024, 16, 64
HPC = 8            # heads per core
HD = HPC * DH      # 512 head dims per core
NCORES = 8
BF16 = ml_dtypes.bfloat16

TOK_TILE = 128     # k-token tile (partition dim of S^T)
QCHUNK = 512       # q-token chunk (free dim of S^T)
NKT = N // TOK_TILE       # 16 k tiles
NQC = N // QCHUNK         # 4 q chunks
NQT = N // 128            # 16 q tiles (out-projection)
DCH = DIM // 128          # 8 contraction chunks over DIM
VROW = HPC * (DH + 1)     # 520: V_aug row elems per k-tile

_CACHE = {}


def _build_program():
    from contextlib import ExitStack
    import concourse.bass as bass
    import concourse.tile as tile
    from concourse import bacc, mybir

    dt = mybir.dt
    f32 = dt.float32
    bf16 = dt.bfloat16
    Exp = mybir.ActivationFunctionType.Exp
    Copy = mybir.ActivationFunctionType.Copy

    nc = bacc.Bacc("TRN2", target_bir_lowering=False, debug=False,
                   enable_asserts=False, num_devices=NCORES)

    xT = nc.dram_tensor("xT", [DIM, N], bf16, kind="ExternalInput").ap()
    wq = nc.dram_tensor("wq", [DIM, HD], bf16, kind="ExternalInput").ap()
    wk = nc.dram_tensor("wk", [DIM, HD], bf16, kind="ExternalInput").ap()
    wv = nc.dram_tensor("wv", [DIM, HD], bf16, kind="ExternalInput").ap()
    wo = nc.dram_tensor("wo", [HD, DIM], bf16, kind="ExternalInput").ap()
    kpm = nc.dram_tensor("kpm", [N, 1], f32, kind="ExternalInput").ap()
    cmask_d = nc.dram_tensor("cmask", [4 * 128, QCHUNK], bf16,
                             kind="ExternalInput").ap()
    out_d = nc.dram_tensor("out", [N, DIM], f32, kind="ExternalOutput").ap()

    with tile.TileContext(nc) as tc, ExitStack() as ctx:
        const = ctx.enter_context(tc.tile_pool(name="const", bufs=1))
        p_sbp = ctx.enter_context(tc.tile_pool(name="p_sbp", bufs=17))
        miscp = ctx.enter_context(tc.tile_pool(name="miscp", bufs=3))
        outp = ctx.enter_context(tc.tile_pool(name="outp", bufs=3))
        mm_ps = ctx.enter_context(tc.tile_pool(name="mm_ps", bufs=2, space="PSUM"))
        s_ps = ctx.enter_context(tc.tile_pool(name="s_ps", bufs=2, space="PSUM"))
        o_ps = ctx.enter_context(tc.tile_pool(name="o_ps", bufs=2, space="PSUM"))

        # ---- persistent SBUF tensors ----
        xT_sb = [const.tile([128, N], bf16, name=f"xTsb{c}") for c in range(DCH)]
        wq_sb = [const.tile([128, HD], bf16, name=f"wqsb{c}") for c in range(DCH)]
        wk_sb = [const.tile([128, HD], bf16, name=f"wksb{c}") for c in range(DCH)]
        wv_sb = [const.tile([128, HD], bf16, name=f"wvsb{c}") for c in range(DCH)]
        wo_sb = [const.tile([128, DIM], bf16, name=f"wosb{c}") for c in range(4)]
        # Q^T / K^T packed: chunk c holds heads 2c (parts 0-63) and 2c+1 (64-127)
        QT = [const.tile([128, N], bf16, name=f"QTsb{c}") for c in range(4)]
        KT = [const.tile([128, N], bf16, name=f"KTsb{c}") for c in range(4)]
        # V_aug: per k-tile block of 8*(64+1) cols
        V = const.tile([128, NKT * VROW], bf16, name="Vsb")
        # O^T packed like QT/KT
        OT = [const.tile([128, N], bf16, name=f"OTsb{c}") for c in range(4)]
        cmask = const.tile([128, 4 * QCHUNK], bf16, name="cmasksb")
        # key-padding mask: col t = mask[t*128 + p]
        kpm_sb = const.tile([128, NKT], f32, name="kpmsb")
        warm = const.tile([128, 512], bf16, name="warm")

        sync = nc.sync
        sync.dma_start(
            kpm_sb.rearrange("p (t one) -> p t one", one=1),
            kpm.rearrange("(t p) one -> p t one", p=128),
        )

        # ---- PE warm-up: ramp the clock out of the low p-state while the
        # input DMAs run.  Reads a zeroed SBUF tile, output never consumed.
        nc.vector.memset(warm[:], 0)
        wps = s_ps.tile([128, 512], f32, tag="s", name="warmps")
        for i in range(20):
            nc.tensor.matmul(wps[:], warm[:, 0:128], warm[:],
                             start=(i == 0), stop=(i == 19))

        # ---- input DMAs, finely staged so qk(0,0) unblocks at ~4us:
        # xT quarter 0 + wq/wk col-block 0 first, then wv (for v units),
        # then the remaining blocks in need order ----
        for c in range(DCH):
            sync.dma_start(xT_sb[c][:, 0:512], xT[c * 128:(c + 1) * 128, 0:512])
        for c in range(DCH):
            sync.dma_start(wq_sb[c][:, 0:128], wq[c * 128:(c + 1) * 128, 0:128])
            sync.dma_start(wk_sb[c][:, 0:128], wk[c * 128:(c + 1) * 128, 0:128])
        for c in range(DCH):
            sync.dma_start(wv_sb[c][:], wv[c * 128:(c + 1) * 128, :])
        # cmask DRAM row r*128+k, col q  ->  SBUF part k, col r*512+q
        # (needed by the first diagonal mask-mults at ~8us)
        sync.dma_start(
            cmask.rearrange("p (r q) -> p r q", r=4),
            cmask_d.rearrange("(r p) q -> p r q", p=128),
        )
        for q in range(1, 4):
            for c in range(DCH):
                sync.dma_start(xT_sb[c][:, q * 512:(q + 1) * 512],
                               xT[c * 128:(c + 1) * 128, q * 512:(q + 1) * 512])
        for c in range(DCH):
            sync.dma_start(wq_sb[c][:, 128:512], wq[c * 128:(c + 1) * 128, 128:512])
            sync.dma_start(wk_sb[c][:, 128:512], wk[c * 128:(c + 1) * 128, 128:512])
        for c in range(4):
            sync.dma_start(wo_sb[c][:], wo[c * 128:(c + 1) * 128, :])

        # ---- filler units: one PSUM group each ----
        def v_unit(kt):
            kpm_t = kpm_sb[:, kt:kt + 1]
            ps = mm_ps.tile([128, 512], f32, tag="mm", name="ps")
            for c in range(DCH):
                nc.tensor.matmul(
                    ps[:], xT_sb[c][:, kt * 128:(kt + 1) * 128], wv_sb[c][:],
                    start=(c == 0), stop=(c == DCH - 1))
            vblk = V[:, kt * VROW:(kt + 1) * VROW].rearrange(
                "p (h c) -> p h c", c=DH + 1)
            # data cols, scaled by key-padding mask (ACT: fast PSUM reads)
            nc.scalar.activation(
                vblk[:, :, 0:DH],
                ps.rearrange("p (h c) -> p h c", c=DH),
                Copy, scale=kpm_t[:, 0:1])
            # ones column = mask value (free-dim stride-0 broadcast read)
            nc.vector.tensor_copy(vblk[:, :, DH:DH + 1].squeeze(),
                                  kpm_t[:, 0:1].broadcast_to([128, HPC]))

        def qk_unit(c, tcx, which):
            tsl = slice(tcx * QCHUNK, (tcx + 1) * QCHUNK)
            w_sb, dst = (wq_sb, QT) if which == "q" else (wk_sb, KT)
            ps = mm_ps.tile([128, 512], f32, tag="mm", name="psqk")
            for d in range(DCH):
                nc.tensor.matmul(
                    ps[:], w_sb[d][:, c * 128:(c + 1) * 128],
                    xT_sb[d][:, tsl],
                    start=(d == 0), stop=(d == DCH - 1))
            nc.vector.tensor_copy(dst[c][:, tsl], ps[:])

        def out_unit(qt):
            y_sb = outp.tile([128, DIM], f32, tag="y", name="y_sb")
            for oc in range(2):
                psy = mm_ps.tile([128, 512], f32, tag="mm", name="psy")
                for cc in range(4):
                    nc.tensor.matmul(
                        psy[:], OT[cc][:, qt * 128:(qt + 1) * 128],
                        wo_sb[cc][:, oc * 512:(oc + 1) * 512],
                        start=(cc == 0), stop=(cc == 3))
                nc.vector.tensor_copy(y_sb[:, oc * 512:(oc + 1) * 512], psy[:])
            sync.dma_start(out_d[qt * 128:(qt + 1) * 128, :], y_sb[:])

        def attend_pair(c, qc, fillers=()):
            """Both heads of chunk c over q-chunk qc.  Rounds emitted in pairs
            (4 scores matmuls in 64-row mode, then 4 AV matmuls in 128-row
            mode, AVs lagged one round-pair); proj filler units interleaved."""
            hA, hB = 2 * c, 2 * c + 1
            ktA, ktB = KT[c][0:64, :], KT[c][64:128, :]
            qtA, qtB = QT[c][0:64, :], QT[c][64:128, :]
            qsl = slice(qc * QCHUNK, (qc + 1) * QCHUNK)
            psoA = o_ps.tile([DH + 1, 512], f32, tag="o", name="psoA")
            psoB = o_ps.tile([DH + 1, 512], f32, tag="o", name="psoB")
            nkt = 4 * qc + 4
            fillers = list(fillers)
            n_rp = nkt // 2

            def emit_scores(kt):
                off = max(0, (kt - 4 * qc) * 128)
                ps = s_ps.tile([128, 1024], f32, tag="s", name="ps")
                nc.tensor.matmul(
                    ps[:, off:512],
                    ktA[:, kt * 128:(kt + 1) * 128],
                    qtA[:, qc * QCHUNK + off:(qc + 1) * QCHUNK],
                    start=True, stop=True)
                nc.tensor.matmul(
                    ps[:, 512 + off:1024],
                    ktB[:, kt * 128:(kt + 1) * 128],
                    qtB[:, qc * QCHUNK + off:(qc + 1) * QCHUNK],
                    start=True, stop=True)
                p2 = p_sbp.tile([128, 1024], bf16, tag="p", name="p2")
                if off > 0:
                    # diagonal round: exp only the live columns of both halves
                    ps3 = ps.rearrange("p (h q) -> p h q", h=2)[:, :, off:512]
                    p23 = p2.rearrange("p (h q) -> p h q", h=2)[:, :, off:512]
                    nc.scalar.activation(p23, ps3, Exp)
                else:
                    nc.scalar.activation(p2[:], ps[:], Exp)
                if kt >= 4 * qc:
                    r = kt - 4 * qc
                    cm = cmask[:, r * QCHUNK + off:(r + 1) * QCHUNK]
                    p23 = p2.rearrange("p (h q) -> p h q", h=2)[:, :, off:512]
                    nc.vector.tensor_mul(
                        p23, p23,
                        cm.rearrange("p (h q) -> p h q", h=1)
                          .broadcast_to([128, 2, 512 - off]))
                return p2

            def emit_av(kt, p2, pso, h, half):
                off = max(0, (kt - 4 * qc) * 128)
                st, sp = (kt == 0), (kt == nkt - 1)
                nc.tensor.matmul(
                    pso[:, off:512],
                    V[:, kt * VROW + h * (DH + 1):
                       kt * VROW + (h + 1) * (DH + 1)],
                    p2[:, half * 512 + off:half * 512 + 512],
                    start=st, stop=sp, skip_group_check=True)

            def normalize(pso, po):
                # O^T[0:64] * (1 / rowsum row 64)
                rs = miscp.tile([1, 512], f32, tag="rs", name="rs")
                nc.vector.tensor_copy(rs[:], pso[DH:DH + 1, :])
                rc = miscp.tile([1, 512], f32, tag="rc", name="rc")
                nc.vector.reciprocal_approx_fast(rc[:], rs[:])
                bc = miscp.tile([64, 512], f32, tag="bc", name="bc")
                nc.gpsimd.partition_broadcast(bc[:], rc[:])
                if po == 0:
                    nc.vector.tensor_mul(OT[c][0:64, qsl],
                                         pso[0:DH, :], bc[:])
                else:
                    otmp = miscp.tile([64, 512], bf16, tag="otmp", bufs=3,
                                      name="otmp")
                    nc.vector.tensor_mul(otmp[:], pso[0:DH, :], bc[:])
                    # partition shift 0->64 needs a DMA, engines can't shift
                    sync.dma_start(OT[c][64:128, qsl], otmp[:])

            # qc-phase batching: ALL scores rounds (one 64-row-mode phase,
            # paced by ACT exp, fillers absorb the PE slack), then ALL AV
            # matmuls (one 128-row-mode phase) -> few mode switches per qc.
            # Head A's AVs complete first so its normalize chain (DVE/gpsimd)
            # hides under head B's AV stream.
            pending = []
            done_f = 0
            for kt in range(nkt):
                pending.append((kt, emit_scores(kt)))
                want = (kt + 1) * len(fillers) // nkt
                while done_f < want:
                    fillers[done_f]()
                    done_f += 1
            # B first: its longer normalize chain (extra DMA partition-shift)
            # hides under A's AV stream; A's shorter chain is tail-exposed.
            for kt, p2 in pending:
                emit_av(kt, p2, psoB, hB, 1)
            normalize(psoB, 64)
            for kt, p2 in pending:
                emit_av(kt, p2, psoA, hA, 0)
            normalize(psoA, 0)

        # ---- global schedule: attention starts after qk(0,0)+v(0..3);
        # all remaining proj groups ride as filler inside attention ----
        F = {}
        for cc in range(4):
            for t in range(NQC):
                F[f"q{cc}{t}"] = (lambda cc=cc, t=t: qk_unit(cc, t, "q"))
                F[f"k{cc}{t}"] = (lambda cc=cc, t=t: qk_unit(cc, t, "k"))
        for kt in range(NKT):
            F[f"v{kt}"] = (lambda kt=kt: v_unit(kt))
        for qt in range(NQT):
            F[f"o{qt}"] = (lambda qt=qt: out_unit(qt))

        qk_unit(0, 0, "q"); qk_unit(0, 0, "k")
        for kt in range(4):
            v_unit(kt)

        plan = {
            (0, 0): ["v4", "v5", "v6", "v7", "q01", "k01"],
            (0, 1): ["v8", "v9", "v10", "v11", "q02", "k02"],
            (0, 2): ["v12", "v13", "v14", "v15", "q03", "k03"],
            (0, 3): ["q10", "k10", "q11", "k11"],
            (1, 0): ["q12", "k12"],
            (1, 1): ["q13", "k13"],
            (1, 2): ["q20", "k20", "q21", "k21"],
            (1, 3): ["q22", "k22", "q23", "k23"],
            (2, 0): [],
            (2, 1): ["q30", "k30"],
            (2, 2): ["q31", "k31", "q32", "k32"],
            (2, 3): ["q33", "k33"],
            (3, 0): [],
            (3, 1): ["o0", "o1", "o2", "o3"],
            (3, 2): ["o4", "o5", "o6", "o7"],
            (3, 3): ["o8", "o9", "o10", "o11"],
        }
        for c in range(4):
            for qc in range(NQC):
                attend_pair(c, qc, [F[n] for n in plan[(c, qc)]])
        for qt in range(12, 16):
            out_unit(qt)

    nc.compile()
    return nc


def _get_program():
    if "nc" not in _CACHE:
        _CACHE["nc"] = _build_program()
    return _CACHE["nc"]


def _prep_inputs(x, mask, w_qkv, w_out):
    """Build the 8 per-core input maps (host-side sharding)."""
    scale = DH ** -0.5
    # causal keep-mask patterns for the 4 diagonal k-tiles of a 512 q-chunk
    k_idx = np.arange(128)[:, None]
    q_idx = np.arange(QCHUNK)[None, :]
    cm = np.concatenate(
        [(q_idx >= r * 128 + k_idx) for r in range(4)], axis=0
    ).astype(BF16)  # [512, 512]

    xT = [np.ascontiguousarray(x[b].T).astype(BF16) for b in range(B)]
    in_maps = []
    for core in range(NCORES):
        b, hg = core // 2, core % 2
        cs = slice(hg * HD, (hg + 1) * HD)
        wq_s = (w_qkv[:, 0 * DIM:1 * DIM][:, cs] * scale).astype(BF16)
        wk_s = w_qkv[:, 1 * DIM:2 * DIM][:, cs].astype(BF16)
        wv_s = w_qkv[:, 2 * DIM:3 * DIM][:, cs].astype(BF16)
        wo_s = np.ascontiguousarray(w_out[cs, :]).astype(BF16)
        kpm = mask[b].astype(np.float32).reshape(N, 1)
        in_maps.append({
            "xT": xT[b], "wq": wq_s, "wk": wk_s, "wv": wv_s, "wo": wo_s,
            "kpm": np.ascontiguousarray(kpm), "cmask": cm,
        })
    return in_maps


def kernel(x, mask, w_qkv, w_out, b_out, _trace=False):
    from concourse import bass_utils

    x = np.asarray(x, dtype=np.float32)
    mask = np.asarray(mask)
    w_qkv = np.asarray(w_qkv, dtype=np.float32)
    w_out = np.asarray(w_out, dtype=np.float32)
    b_out = np.asarray(b_out, dtype=np.float32)

    nc = _get_program()
    in_maps = _prep_inputs(x, mask, w_qkv, w_out)
    res = bass_utils.run_bass_kernel_spmd(
        nc, in_maps, core_ids=list(range(NCORES)), trace=_trace)

    out = np.empty((B, N, DIM), dtype=np.float32)
    for b in range(B):
        out[b] = res.results[2 * b]["out"] + res.results[2 * b + 1]["out"] + b_out
    if _trace:
        return out, res
    return out
